# revision 3
# baseline (speedup 1.0000x reference)
"""BertSelfAttention (B=4, S=2048, H=1024, NH=16, HD=64) on 8 Trainium2 NeuronCores.

Sharding: batch (4) x head-group (2) -> 8 cores. Core c handles batch b=c//2 and
heads [g*8, g*8+8) with g=c%2 (output channels [g*512, (g+1)*512)).

Per-core math (all on device):
  QT[ch, s] = (wq_c @ x_b^T + bq_c),  KT likewise       (channels on partitions)
  V[s, ch]  = (x_b @ wv_c^T + bv_c)                     (tokens on partitions)
  per head h: scoresT[j, i] = KT_h^T-contracted matmul (K=64)
              expT = exp(scoresT/8 + mask_j)            (mask is per-partition bias)
              ctxT[d, i] (+ denom row via fused ones-column in V) accumulated over j
              out = ctxT[0:64] / denom                  (reciprocal + outer-product bcast)
Output per core: ctxT [512, 2048]; host transposes into [B, S, H].

Matmuls run as float32r (full-rate fp32 with hardware rounding, ~2e-4 rel err).
"""

import sys

if "/opt/trn_rl_repo" not in sys.path:
    sys.path.insert(0, "/opt/trn_rl_repo")

import numpy as np

B, S, H = 4, 2048, 1024
NH, HD = 16, 64
HPC = 8          # heads per core
CH = HPC * HD    # 512 output channels per core
CT = H // 128    # 8 contraction tiles
JT = CH // 128   # 4 channel tiles per core
ST = S // 128    # 16 token tiles
IC = S // 512    # 4 i-chunks
VW = HD + 1      # 65: v columns + fused ones column

_CACHE = {}


def _build():
    import concourse.bass as bass  # noqa: F401  (registers engine methods)
    import concourse.mybir as mybir
    import concourse.tile as tile
    from concourse import bacc

    F32 = mybir.dt.float32
    F32R = mybir.dt.float32r

    nc = bacc.Bacc("TRN2", target_bir_lowering=False, debug=True)

    xt = nc.dram_tensor("xt", [H, S], F32, kind="ExternalInput")        # x_b^T
    wq_t = nc.dram_tensor("wq_t", [H, CH], F32, kind="ExternalInput")   # wq_c^T
    wk_t = nc.dram_tensor("wk_t", [H, CH], F32, kind="ExternalInput")
    wv_t = nc.dram_tensor("wv_t", [H, CH], F32, kind="ExternalInput")
    bq = nc.dram_tensor("bq", [CH], F32, kind="ExternalInput")
    bk = nc.dram_tensor("bk", [CH], F32, kind="ExternalInput")
    bv = nc.dram_tensor("bv", [CH], F32, kind="ExternalInput")
    mask = nc.dram_tensor("mask", [S], F32, kind="ExternalInput")
    ones = nc.dram_tensor("ones", [512], F32, kind="ExternalInput")
    out = nc.dram_tensor("out", [CH, S], F32, kind="ExternalOutput")    # ctxT

    with tile.TileContext(nc) as tc, nc.allow_low_precision(reason="fp32r attention"):
        from contextlib import ExitStack

        with ExitStack() as outer:
            persist = outer.enter_context(tc.tile_pool(name="persist", bufs=1))

            # Persistent SBUF tensors (live across both phases)
            qt_sb = [persist.tile([128, S], F32R, tag=f"qt{j}", name=f"qt{j}") for j in range(JT)]
            kt_sb = [persist.tile([128, S], F32R, tag=f"kt{j}", name=f"kt{j}") for j in range(JT)]
            v_sb = persist.tile([128, ST, VW * HPC], F32R, tag="v")
            mask_sb = persist.tile([128, ST], F32, tag="mask")
            bq_sb = persist.tile([1, CH], F32R, tag="bq")
            bk_sb = persist.tile([1, CH], F32R, tag="bk")
            bv_sb = persist.tile([1, CH], F32R, tag="bv")
            ones_r = persist.tile([1, 512], F32R, tag="ones_r")
            ones_f = persist.tile([1, 64], F32, tag="ones_f")

            nc.sync.dma_start(out=mask_sb, in_=mask.rearrange("(t p) -> p t", p=128))
            nc.sync.dma_start(out=bq_sb, in_=bq[None, :].bitcast(F32R))
            nc.sync.dma_start(out=bk_sb, in_=bk[None, :].bitcast(F32R))
            nc.sync.dma_start(out=bv_sb, in_=bv[None, :].bitcast(F32R))
            nc.sync.dma_start(out=ones_r, in_=ones[None, :].bitcast(F32R))
            nc.sync.dma_start(out=ones_f, in_=ones[None, 0:64])
            # ones columns of v (position 64 of each head block, every token tile)
            v4 = v_sb.rearrange("p t (h e) -> p t h e", e=VW)
            ones_bcast = bass.AP(
                tensor=ones.bitcast(F32R),
                offset=0,
                ap=[[0, 128], [1, HPC]],
            )
            for t in range(ST):
                nc.sync.dma_start(out=v4[:, t, :, HD], in_=ones_bcast)

            # ---------------- Phase 1: QKV projections ----------------
            with ExitStack() as ph1:
                wpool = ph1.enter_context(tc.tile_pool(name="w", bufs=1))
                xpool = ph1.enter_context(tc.tile_pool(name="x", bufs=3))
                ppool = ph1.enter_context(tc.tile_pool(name="pp", bufs=1, space="PSUM"))

                wq_sb = wpool.tile([128, CT, CH], F32R, tag="wq")
                wk_sb = wpool.tile([128, CT, CH], F32R, tag="wk")
                wv_sb = wpool.tile([128, CT, CH], F32R, tag="wv")
                nc.sync.dma_start(
                    out=wq_sb, in_=wq_t.rearrange("(c p) j -> p c j", p=128).bitcast(F32R))
                nc.sync.dma_start(
                    out=wk_sb, in_=wk_t.rearrange("(c p) j -> p c j", p=128).bitcast(F32R))
                nc.sync.dma_start(
                    out=wv_sb, in_=wv_t.rearrange("(c p) j -> p c j", p=128).bitcast(F32R))

                xt_r = xt.rearrange("(c p) s -> c p s", p=128).bitcast(F32R)

                # Q and K passes: channels on psum partitions
                for name, w_sb, b_sb, dst in (
                    ("q", wq_sb, bq_sb, qt_sb),
                    ("k", wk_sb, bk_sb, kt_sb),
                ):
                    for sh in range(2):  # halves of the token axis
                        ps = [
                            [ppool.tile([128, 512], F32, tag=f"pp{j}{sc}",
                                        name=f"ps_{name}{sh}{j}{sc}")
                             for sc in range(2)]
                            for j in range(JT)
                        ]
                        for ct in range(CT):
                            x_t = xpool.tile([128, 1024], F32R, tag="x")
                            nc.sync.dma_start(
                                out=x_t, in_=xt_r[ct, :, sh * 1024:(sh + 1) * 1024])
                            for j in range(JT):
                                for sc in range(2):
                                    nc.tensor.matmul(
                                        ps[j][sc],
                                        lhsT=w_sb[:, ct, j * 128:(j + 1) * 128],
                                        rhs=x_t[:, sc * 512:(sc + 1) * 512],
                                        start=(ct == 0), stop=False)
                        for j in range(JT):
                            for sc in range(2):
                                nc.tensor.matmul(
                                    ps[j][sc],
                                    lhsT=b_sb[:, j * 128:(j + 1) * 128],
                                    rhs=ones_r,
                                    start=False, stop=True)
                                nc.vector.tensor_copy(
                                    dst[j][:, sh * 1024 + sc * 512:
                                           sh * 1024 + (sc + 1) * 512],
                                    ps[j][sc])

                # V pass: tokens on psum partitions
                for sh in range(2):
                    ps = [ppool.tile([128, 512], F32, tag=f"pp{j}{sc}",
                                     name=f"ps_v{sh}{j}{sc}")
                          for j in range(JT) for sc in range(2)]
                    for ct in range(CT):
                        x_t = xpool.tile([128, 1024], F32R, tag="x")
                        nc.sync.dma_start(
                            out=x_t, in_=xt_r[ct, :, sh * 1024:(sh + 1) * 1024])
                        for st in range(8):
                            nc.tensor.matmul(
                                ps[st],
                                lhsT=x_t[:, st * 128:(st + 1) * 128],
                                rhs=wv_sb[:, ct, :],
                                start=(ct == 0), stop=False)
                    for st in range(8):
                        nc.tensor.matmul(
                            ps[st],
                            lhsT=ones_r[:, 0:128],
                            rhs=bv_sb,
                            start=False, stop=True)
                        for h in range(HPC):
                            nc.vector.tensor_copy(
                                v_sb[:, sh * 8 + st, h * VW:h * VW + HD],
                                ps[st][:, h * HD:(h + 1) * HD])

            # ---------------- Phase 2: attention ----------------
            with ExitStack() as ph2:
                spool = ph2.enter_context(tc.tile_pool(name="sp", bufs=3, space="PSUM"))
                cpool = ph2.enter_context(tc.tile_pool(name="cp", bufs=2, space="PSUM"))
                bpool = ph2.enter_context(tc.tile_pool(name="bp", bufs=2, space="PSUM"))
                epool = ph2.enter_context(tc.tile_pool(name="ep", bufs=3))
                opool = ph2.enter_context(tc.tile_pool(name="op", bufs=3))

                for h in range(HPC):
                    qi, po = h // 2, (h % 2) * 64
                    for ic in range(IC):
                        ctx_ps = cpool.tile([VW, 512], F32, tag="ctx")
                        for st in range(ST):
                            s_ps = spool.tile([128, 512], F32, tag="sc")
                            nc.tensor.matmul(
                                s_ps,
                                lhsT=kt_sb[qi][po:po + 64, st * 128:(st + 1) * 128],
                                rhs=qt_sb[qi][po:po + 64, ic * 512:(ic + 1) * 512],
                                start=True, stop=True)
                            e_sb = epool.tile([128, 512], F32R, tag="e")
                            nc.scalar.activation(
                                e_sb, s_ps,
                                mybir.ActivationFunctionType.Exp,
                                bias=mask_sb[:, st:st + 1], scale=0.125)
                            nc.tensor.matmul(
                                ctx_ps,
                                lhsT=v_sb[:, st, h * VW:(h + 1) * VW],
                                rhs=e_sb,
                                start=(st == 0), stop=(st == ST - 1))
                        r_sb = opool.tile([1, 512], F32, tag="r")
                        nc.vector.reciprocal(r_sb, ctx_ps[64:65, :])
                        bc_ps = bpool.tile([64, 512], F32, tag="bc")
                        nc.tensor.matmul(
                            bc_ps, lhsT=ones_f, rhs=r_sb, start=True, stop=True)
                        bc_sb = opool.tile([64, 512], F32, tag="bcs")
                        nc.vector.tensor_copy(bc_sb, bc_ps)
                        o_sb = opool.tile([64, 512], F32, tag="o")
                        nc.vector.tensor_mul(o_sb, ctx_ps[0:64, :], bc_sb)
                        nc.sync.dma_start(
                            out=out[h * 64:(h + 1) * 64, ic * 512:(ic + 1) * 512],
                            in_=o_sb)

    nc.compile()
    return nc


def _get_nc():
    if "nc" not in _CACHE:
        _CACHE["nc"] = _build()
    return _CACHE["nc"]


def _in_maps(hidden_states, attention_mask, wq, bq, wk, bk, wv, bv):
    ones = np.ones(512, np.float32)
    maps = []
    for c in range(8):
        b, g = c // 2, c % 2
        ch0 = g * CH
        maps.append({
            "xt": np.ascontiguousarray(hidden_states[b].T),
            "wq_t": np.ascontiguousarray(wq[ch0:ch0 + CH, :].T),
            "wk_t": np.ascontiguousarray(wk[ch0:ch0 + CH, :].T),
            "wv_t": np.ascontiguousarray(wv[ch0:ch0 + CH, :].T),
            "bq": np.ascontiguousarray(bq[ch0:ch0 + CH]),
            "bk": np.ascontiguousarray(bk[ch0:ch0 + CH]),
            "bv": np.ascontiguousarray(bv[ch0:ch0 + CH]),
            "mask": np.ascontiguousarray(attention_mask[b, 0, 0, :]),
            "ones": ones,
        })
    return maps


def _gather(results):
    full = np.empty((B, S, H), np.float32)
    for c in range(8):
        b, g = c // 2, c % 2
        full[b, :, g * CH:(g + 1) * CH] = results[c]["out"].T
    return full


def _run(in_maps, trace=False):
    from concourse.bass_utils import run_bass_kernel_spmd

    nc = _get_nc()
    return run_bass_kernel_spmd(nc, in_maps, list(range(8)), trace=trace)


def kernel(hidden_states, attention_mask, wq, bq, wk, bk, wv, bv):
    args = [np.asarray(a, np.float32) for a in
            (hidden_states, attention_mask, wq, bq, wk, bk, wv, bv)]
    res = _run(_in_maps(*args))
    return _gather(res.results)


def kernel_profiled(hidden_states, attention_mask, wq, bq, wk, bk, wv, bv):
    """Like kernel() but with NTFF tracing; returns (output, exec_time_ns)."""
    args = [np.asarray(a, np.float32) for a in
            (hidden_states, attention_mask, wq, bq, wk, bk, wv, bv)]
    res = _run(_in_maps(*args), trace=True)
    return _gather(res.results), res.exec_time_ns
